# revision 1
# baseline (speedup 1.0000x reference)
"""Cumulative LayerNorm Trainium2 Bass kernel.

x: [B=8, C=256, T=16000] f32.  Per timestep t: normalize x[:, :, t] by the
mean/std of all elements x[:, :, t'<=t] (cumulative over channels+time), then
scale by weight[c] and add bias[c].

Sharding: pure data parallel over B across 8 NeuronCores (1 sample/core).

Per-core algorithm (C=256 = 2 halves of 128 partitions, T on the free dim):
  Phase A (per 2000-col io-tile):
    - DMA x into SBUF (labeled f32r so the PE consumes it directly; fp32r
      truncates operands to ~13 mantissa bits inside the PE only).
    - xx = x^2 in bf16 (ACT for half 0, GPSIMD for half 1).
    - PE: s[t] = sum_c x (fp32r, exact ones weights) and sq[t] = sum_c x^2
      (bf16) as [2, 2, 512] PSUM row-blocks; evacuate row 0 to SBUF rows
      (ACT copies); DMA-reshape rows into a [128, 125] "stat layout" where
      t = 125*p + i.
  Stats (per 4000-col chunk = 32 stat rows; engine ops need 32-aligned
  partition bases):
    - DVE tensor_tensor_scan along i (per-partition prefix sums of the
      chunk's 32 rows).
    - Row totals accumulate into st[128, 2]; a strict-upper-triangular fp32r
      matmul gives exclusive cross-partition offsets (st rows of future
      chunks are zeroed, so one full-K matmul per chunk is exact).
    - mean = (scan + off) * 1/cnt (off read straight from PSUM);
      var = E[x^2] - mean^2;  istd = 1/sqrt(var + eps) (ACT Sqrt + DVE
      reciprocal);  nm = -mean.
  Phase C (per io-tile, per 1000-col half-tile):
    - Gather nm/istd stat-layout slices back into [1, 1000] rows (DMA).
    - PE rank-1 broadcasts: nm_bc = ones x nm_row (PSUM),
      ibc = ones x istd_row (PSUM, copied to SBUF on ACT).
    - DVE scalar_tensor_tensor pair per half: z = nm_bc + x;
      y = (z * w[p]) * istd_bc; DMA out.

Emission is software-pipelined at io-tile granularity (phase C lags phase A
by 3 tiles) so the strict-FIFO engine queues always hold ready work ahead of
the long-latency stats chain.
"""
import ml_dtypes
import numpy as np

B, C, T = 8, 256, 16000
P = 128
NH = 2                     # channel halves
CHUNK = 2000               # t per io-tile
NCHUNK = T // CHUNK        # 8
ROWS = T // P              # 125  (stat layout free dim; t = 125*p + i)
PB = 500                   # psum block columns (4 per io-tile)
NPB = CHUNK // PB          # 4
EPS = 1e-06

_cached = {}


def _build_nc(with_bias: bool):
    from contextlib import ExitStack

    import concourse.tile as tile
    from concourse import bacc, mybir

    f32 = mybir.dt.float32
    f32r = mybir.dt.float32r
    bf16 = mybir.dt.bfloat16
    ALU = mybir.AluOpType
    ACTF = mybir.ActivationFunctionType

    nc = bacc.Bacc()

    x = nc.dram_tensor("x", [C, T], f32, kind="ExternalInput")
    wvec = nc.dram_tensor("wvec", [C, 1], f32, kind="ExternalInput")
    iden_d = nc.dram_tensor("iden", [P, P], f32r, kind="ExternalInput")
    tri_d = nc.dram_tensor("tri", [P, P], f32r, kind="ExternalInput")
    ones2r_d = nc.dram_tensor("ones2r", [P, 2], f32r, kind="ExternalInput")
    ones2b_d = nc.dram_tensor("ones2b", [P, 2], bf16, kind="ExternalInput")
    onesb_d = nc.dram_tensor("onesb", [1, P], f32r, kind="ExternalInput")
    zeros2_d = nc.dram_tensor("zeros2", [P, 2], f32r, kind="ExternalInput")
    invcnt_d = nc.dram_tensor("invcnt", [P, ROWS], f32, kind="ExternalInput")
    if with_bias:
        bvec = nc.dram_tensor("bvec", [C, 1], f32, kind="ExternalInput")
    y = nc.dram_tensor("y", [C, T], f32, kind="ExternalOutput")

    with tile.TileContext(nc) as tc, ExitStack() as ctx:
        const = ctx.enter_context(tc.tile_pool(name="const", bufs=1))
        persist = ctx.enter_context(tc.tile_pool(name="persist", bufs=1))
        xpool = ctx.enter_context(tc.tile_pool(name="xpool", bufs=6))
        ypool = ctx.enter_context(tc.tile_pool(name="ypool", bufs=2))
        sqpool = ctx.enter_context(tc.tile_pool(name="sqpool", bufs=2))
        erow = ctx.enter_context(tc.tile_pool(name="erow", bufs=4))
        brow = ctx.enter_context(tc.tile_pool(name="brow", bufs=4))
        ibcsb = ctx.enter_context(tc.tile_pool(name="ibcsb", bufs=4))
        ps_s = ctx.enter_context(tc.tile_pool(name="ps_s", bufs=2, space="PSUM"))
        ps_nm = ctx.enter_context(tc.tile_pool(name="ps_nm", bufs=1, space="PSUM"))
        ps_i = ctx.enter_context(tc.tile_pool(name="ps_i", bufs=1, space="PSUM"))
        zpool = ctx.enter_context(tc.tile_pool(name="zpool", bufs=3))

        # ---- constants ----
        tri = const.tile([P, P], f32r)
        nc.sync.dma_start(out=tri, in_=tri_d[:, :])
        ones2r = const.tile([P, 2], f32r)
        nc.sync.dma_start(out=ones2r, in_=ones2r_d[:, :])
        ones2b = const.tile([P, 2], bf16)
        nc.sync.dma_start(out=ones2b, in_=ones2b_d[:, :])
        onesb = const.tile([1, P], f32r)
        nc.sync.dma_start(out=onesb, in_=onesb_d[:, :])
        invcnt = const.tile([P, ROWS], f32)
        nc.sync.dma_start(out=invcnt, in_=invcnt_d[:, :])
        w_sb = const.tile([P, NH], f32)
        for h in range(NH):
            nc.sync.dma_start(out=w_sb[:, h : h + 1], in_=wvec[h * P : (h + 1) * P, 0:1])
        if with_bias:
            b_sb = const.tile([P, NH], f32)
            for h in range(NH):
                nc.sync.dma_start(
                    out=b_sb[:, h : h + 1], in_=bvec[h * P : (h + 1) * P, 0:1]
                )
        eps_sb = const.tile([P, 1], f32)
        nc.vector.memset(eps_sb, EPS)

        # ---- persistent stat-layout surfaces ----
        s_re = persist.tile([P, ROWS], f32)     # channel sums -> prefix sums
        sq_re = persist.tile([P, ROWS], f32)
        mean_t = persist.tile([P, ROWS], f32)
        ex2_t = persist.tile([P, ROWS], f32)    # E[x^2] -> var
        msq_t = persist.tile([P, ROWS], f32)    # mean^2 -> ln(var+eps)
        istd_t = persist.tile([P, ROWS], f32)
        nm_t = persist.tile([P, ROWS], f32)     # -mean
        st_sb = persist.tile([P, 2], f32r)      # chunk totals (s, sq)
        nc.sync.dma_start(out=st_sb, in_=zeros2_d[:, :])

        def phase_a(tix, x_t):
            """Load io-tile `tix` (2000 cols), compute channel sums/sumsq into
            stat-layout rows 16*tix .. 16*tix+16."""
            t0 = tix * CHUNK
            for h in range(NH):
                nc.sync.dma_start(
                    out=x_t[:, h, :],
                    in_=x[h * P : (h + 1) * P, t0 : t0 + CHUNK].bitcast(f32r),
                )
            xx0 = sqpool.tile([P, CHUNK], bf16, tag="xx0", name="xx0")
            nc.scalar.activation(xx0, x_t[:, 0, :].bitcast(f32), ACTF.Square)
            xx1 = sqpool.tile([P, CHUNK], bf16, tag="xx1", name="xx1")
            nc.gpsimd.tensor_tensor(
                xx1, x_t[:, 1, :].bitcast(f32), x_t[:, 1, :].bitcast(f32), ALU.mult
            )

            for a2 in range(2):  # 1000-col groups
                sps = ps_s.tile([2, 2, 512], f32, tag="stat", name="sps")
                qps = ps_s.tile([2, 2, 512], f32, tag="stat", name="qps")
                for j in range(2):
                    cs = slice((2 * a2 + j) * PB, (2 * a2 + j + 1) * PB)
                    nc.tensor.matmul(
                        sps[0:2, j, 0:PB], ones2r, x_t[:, 0, cs],
                        start=True, stop=False,
                    )
                    nc.tensor.matmul(
                        sps[0:2, j, 0:PB], ones2r, x_t[:, 1, cs],
                        start=False, stop=True,
                    )
                for j in range(2):
                    cs = slice((2 * a2 + j) * PB, (2 * a2 + j + 1) * PB)
                    nc.tensor.matmul(
                        qps[0:2, j, 0:PB], ones2b, xx0[:, cs], start=True, stop=False
                    )
                    nc.tensor.matmul(
                        qps[0:2, j, 0:PB], ones2b, xx1[:, cs], start=False, stop=True
                    )
                srow = erow.tile([1, 1024], f32, tag="erow", name="srow")
                nc.scalar.copy(
                    srow[0:1, 0:1000].rearrange("p (j n) -> p j n", j=2),
                    sps[0:1, :, 0:PB],
                )
                qrow = erow.tile([1, 1024], f32, tag="erow", name="qrow")
                nc.scalar.copy(
                    qrow[0:1, 0:1000].rearrange("p (j n) -> p j n", j=2),
                    qps[0:1, :, 0:PB],
                )
                # rows 16*tix+8*a2 .. +8 of the stat layout (t = 125*p + i)
                rp = 16 * tix + 8 * a2
                nc.sync.dma_start(out=s_re[rp : rp + 8, :], in_=srow[0:1, 0:1000])
                nc.sync.dma_start(out=sq_re[rp : rp + 8, :], in_=qrow[0:1, 0:1000])

        def stats(sc):
            """Prefix sums + mean/istd for stat-layout rows 32*sc .. 32*sc+32."""
            sl = slice(32 * sc, 32 * sc + 32)
            nc.vector.tensor_tensor_scan(
                out=s_re[sl, :], data0=s_re[sl, :], data1=s_re[sl, :],
                initial=0.0, op0=ALU.add, op1=ALU.bypass,
            )
            nc.vector.tensor_tensor_scan(
                out=sq_re[sl, :], data0=sq_re[sl, :], data1=sq_re[sl, :],
                initial=0.0, op0=ALU.add, op1=ALU.bypass,
            )
            nc.vector.tensor_copy(st_sb[sl, 0:1], s_re[sl, ROWS - 1 : ROWS])
            nc.vector.tensor_copy(st_sb[sl, 1:2], sq_re[sl, ROWS - 1 : ROWS])
            offps = ps_s.tile([P, 2], f32, tag="stat", name="offps")
            nc.tensor.matmul(offps, tri, st_sb, start=True, stop=True)

            nc.vector.scalar_tensor_tensor(
                out=mean_t[sl, :], in0=s_re[sl, :], scalar=offps[sl, 0:1],
                in1=invcnt[sl, :], op0=ALU.add, op1=ALU.mult,
            )
            nc.vector.scalar_tensor_tensor(
                out=ex2_t[sl, :], in0=sq_re[sl, :], scalar=offps[sl, 1:2],
                in1=invcnt[sl, :], op0=ALU.add, op1=ALU.mult,
            )
            nc.vector.tensor_scalar_mul(nm_t[sl, :], mean_t[sl, :], -1.0)
            nc.vector.tensor_tensor(msq_t[sl, :], mean_t[sl, :], mean_t[sl, :], ALU.mult)
            nc.vector.tensor_tensor(ex2_t[sl, :], ex2_t[sl, :], msq_t[sl, :], ALU.subtract)
            # istd = 1 / sqrt(var + eps)  (Sqrt keeps the ACT table set stable;
            # reciprocal_approx_fast is ~18 bits, far above the fp32r noise)
            nc.scalar.activation(
                msq_t[sl, :], ex2_t[sl, :], ACTF.Sqrt, bias=eps_sb[sl, :], scale=1.0
            )
            nc.vector.reciprocal(out=istd_t[sl, :], in_=msq_t[sl, :])

        def phase_c(tix, x_t):
            """Normalize io-tile `tix` and store it."""
            t0 = tix * CHUNK
            for half in range(2):  # half-tiles of 1000 columns
                rsl = slice(16 * tix + 8 * half, 16 * tix + 8 * half + 8)
                nm_row = brow.tile([1, 1024], f32r, tag="brow", name="nm_row")
                nc.sync.dma_start(
                    out=nm_row[0:1, 0:1000], in_=nm_t[rsl, :].bitcast(f32r)
                )
                istd_row = brow.tile([1, 1024], f32r, tag="brow", name="istd_row")
                nc.sync.dma_start(
                    out=istd_row[0:1, 0:1000], in_=istd_t[rsl, :].bitcast(f32r)
                )
                nm_ps = ps_nm.tile([P, 2, 512], f32, tag="nm", name="nm_ps")
                ibc = ps_i.tile([P, 2, 512], f32, tag="ibc_ps", name="ibc")
                for j in range(2):
                    cs = slice(j * PB, (j + 1) * PB)
                    nc.tensor.matmul(
                        nm_ps[:, j, 0:PB], onesb, nm_row[0:1, cs],
                        start=True, stop=True,
                    )
                    nc.tensor.matmul(
                        ibc[:, j, 0:PB], onesb, istd_row[0:1, cs], start=True, stop=True
                    )
                ibc_sb = ibcsb.tile([P, 2, 512], f32, tag="ibc", name="ibc_sb")
                nc.scalar.copy(ibc_sb[:, :, 0:PB], ibc[:, :, 0:PB])
                for h in range(NH):
                    x_ap = x_t[:, h, half * 1000 : (half + 1) * 1000].bitcast(
                        f32
                    ).rearrange("p (j n) -> p j n", j=2)
                    # z = x - mean  (one DVE op; nm_ps is the -mean broadcast)
                    z_sb = zpool.tile([P, 2, 512], f32, tag="z", name="z_sb")
                    nc.vector.scalar_tensor_tensor(
                        out=z_sb[:, :, 0:PB], in0=nm_ps[:, :, 0:PB], scalar=1.0,
                        in1=x_ap, op0=ALU.mult, op1=ALU.add,
                    )
                    # y = (z * w) * istd
                    y_t = ypool.tile([P, 2, 512], f32, tag="y", name="y_t")
                    nc.vector.scalar_tensor_tensor(
                        out=y_t[:, :, 0:PB], in0=z_sb[:, :, 0:PB],
                        scalar=w_sb[:, h : h + 1], in1=ibc_sb[:, :, 0:PB],
                        op0=ALU.mult, op1=ALU.mult,
                    )
                    if with_bias:
                        nc.vector.tensor_scalar_add(
                            out=y_t[:, :, 0:PB], in0=y_t[:, :, 0:PB],
                            scalar1=b_sb[:, h : h + 1],
                        )
                    nc.sync.dma_start(
                        out=y[h * P : (h + 1) * P,
                              t0 + half * 1000 : t0 + (half + 1) * 1000],
                        in_=y_t[:, :, 0:PB],
                    )

        # Software-pipelined emission, interleaved at io-tile granularity:
        # phase C lags phase A by ~3 tiles and A/C alternate in the emission
        # stream so every engine queue always holds ready work ahead of the
        # long-latency stats chain (strict-FIFO queues otherwise head-of-line
        # block at every chunk boundary).
        tiles = {}

        def load_a(tix):
            x_t = xpool.tile([P, NH, CHUNK], f32r, tag="x", name="x_t")
            phase_a(tix, x_t)
            tiles[tix] = x_t

        na = nc_done = 0
        for tix in range(3):
            load_a(tix)
        na = 3
        while nc_done < NCHUNK:
            if nc_done % 2 == 0:
                stats(nc_done // 2)
            phase_c(nc_done, tiles.pop(nc_done))
            nc_done += 1
            if na < NCHUNK:
                load_a(na)
                na += 1
    nc.compile()
    return nc


def _consts():
    iden = np.eye(P, dtype=np.float32)
    tri = np.triu(np.ones((P, P), dtype=np.float32), k=1)  # tri[k,m]=1 iff k<m
    ones2 = np.ones((P, 2), dtype=np.float32)
    onesb = np.ones((1, P), dtype=np.float32)
    t_idx = (125 * np.arange(P, dtype=np.float64)[:, None]
             + np.arange(ROWS, dtype=np.float64)[None, :])
    invcnt = (1.0 / (C * (t_idx + 1.0))).astype(np.float32)
    return {"iden": iden, "tri": tri, "ones2r": ones2,
            "ones2b": ones2.astype(ml_dtypes.bfloat16), "onesb": onesb,
            "zeros2": np.zeros((P, 2), dtype=np.float32), "invcnt": invcnt}


def _get_nc(with_bias: bool):
    key = ("nc", with_bias)
    if key not in _cached:
        _cached[key] = _build_nc(with_bias)
    return _cached[key]


def _run(x, weight, bias, trace=False):
    from concourse.bass_utils import run_bass_kernel_spmd

    x = np.ascontiguousarray(np.asarray(x, dtype=np.float32))
    weight = np.asarray(weight, dtype=np.float32).reshape(C, 1)
    bias = np.asarray(bias, dtype=np.float32).reshape(C, 1)
    with_bias = bool(np.any(bias))
    nc = _get_nc(with_bias)

    consts = _consts()
    in_maps = []
    for b in range(B):
        m = {"x": np.ascontiguousarray(x[b]), "wvec": weight}
        if with_bias:
            m["bvec"] = bias
        m.update(consts)
        in_maps.append(m)

    res = run_bass_kernel_spmd(nc, in_maps, core_ids=list(range(B)), trace=trace)
    y = np.stack([r["y"] for r in res.results], axis=0)
    return y, res


def kernel(x, weight, bias):
    y, _ = _run(x, weight, bias, trace=False)
    return y



# revision 9
# speedup vs baseline: 1.0793x; 1.0793x over previous
"""Cumulative LayerNorm Trainium2 Bass kernel (v2: single-stats, bf16 path).

x: [B=8, C=256, T=16000] f32.  Per timestep t: normalize x[:, :, t] by the
mean/std of all elements x[:, :, t'<=t] (cumulative over channels+time), then
scale by weight[c] and add bias[c].

Sharding: pure data parallel over B across 8 NeuronCores (1 sample/core).

Per-core plan (C=256 = 2 halves of 128 partitions, T on the free dim).
All 8 io-tiles of x stay resident in SBUF as bf16 (64 KB/partition), so the
kernel runs as two DMA-bound epochs with one tiny serial stats step between:

  Phase A (x8 io-tiles of 2000 cols):
    - gpsimd cast-DMA loads x f32 HBM -> bf16 SBUF (one DMA per tile).
    - xx = x^2 (DVE, bf16 2x mode, one FD-4000 op per tile).
    - PE: per 500-col block one accumulation group of 4 bf16 matmuls with
      half-zero weights ([1100] / [0011] columns) sums s over both channel
      halves into PSUM rows 0-1 and sq into rows 2-3.
    - ACT evacuates [4, 1000] PSUM -> bf16, one DMA scatters it into the
      [128, 250] stat surface (t = 125*p + i; s in cols 0:125, sq 125:250).
  Stats (once):
    - DVE tensor_tensor_scan over the full [128, 125] stat rows (fp32
      accumulator over bf16 terms), strict-upper-triangular f32r matmul for
      exclusive cross-partition offsets, then mean / var / istd / -mean.
  Phase C (x8 io-tiles, per 1000-col half):
    - one [1, 2000] gather DMA per field per tile back into row layout,
      PE rank-1 broadcasts (ones x row) into PSUM, ACT evacuates to bf16.
    - z = x + (-mean) on GPSIMD; y = (z * w[p]) * istd on DVE (bf16 2x).
    - gpsimd cast-DMA store bf16 SBUF -> f32 HBM (one DMA per tile).
"""
import ml_dtypes
import numpy as np

B, C, T = 8, 256, 16000
P = 128
NH = 2                     # channel halves
CHUNK = 2000               # t per io-tile
NCHUNK = T // CHUNK        # 8
ROWS = T // P              # 125  (stat layout free dim; t = 125*p + i)
PB = 500                   # psum block columns
EPS = 1e-06

_cached = {}


def _build_nc(with_bias: bool):
    from contextlib import ExitStack

    import concourse.tile as tile
    from concourse import bacc, mybir

    f32 = mybir.dt.float32
    f32r = mybir.dt.float32r
    bf16 = mybir.dt.bfloat16
    ALU = mybir.AluOpType
    ACTF = mybir.ActivationFunctionType

    nc = bacc.Bacc()

    x = nc.dram_tensor("x", [C, T], f32, kind="ExternalInput")
    wvec = nc.dram_tensor("wvec", [C, 1], f32, kind="ExternalInput")
    tri_d = nc.dram_tensor("tri", [P, P], f32r, kind="ExternalInput")
    oz2_d = nc.dram_tensor("oz2", [P, 2], bf16, kind="ExternalInput")
    zo2_d = nc.dram_tensor("zo2", [P, 2], bf16, kind="ExternalInput")
    onesb_d = nc.dram_tensor("onesb", [1, P], f32r, kind="ExternalInput")
    invcnt_d = nc.dram_tensor("invcnt", [P, ROWS], f32, kind="ExternalInput")
    if with_bias:
        bvec = nc.dram_tensor("bvec", [C, 1], f32, kind="ExternalInput")
    y = nc.dram_tensor("y", [C, T], f32, kind="ExternalOutput")

    # DRAM views with channel halves split onto the partition dim.
    x_v = x.rearrange("(h p) t -> p h t", h=NH)
    y_v = y.rearrange("(h p) t -> p h t", h=NH)

    with tile.TileContext(nc) as tc, ExitStack() as ctx:
        const = ctx.enter_context(tc.tile_pool(name="const", bufs=1))
        persist = ctx.enter_context(tc.tile_pool(name="persist", bufs=1))
        xpool = ctx.enter_context(tc.tile_pool(name="xpool", bufs=NCHUNK))
        sqpool = ctx.enter_context(tc.tile_pool(name="sqpool", bufs=2))
        rowpool = ctx.enter_context(tc.tile_pool(name="rowpool", bufs=3))
        browp = ctx.enter_context(tc.tile_pool(name="browp", bufs=4))
        bcpool = ctx.enter_context(tc.tile_pool(name="bcpool", bufs=4))
        zpool = ctx.enter_context(tc.tile_pool(name="zpool", bufs=3))
        ypool = ctx.enter_context(tc.tile_pool(name="ypool", bufs=2))
        ps_stat = ctx.enter_context(tc.tile_pool(name="ps_stat", bufs=2, space="PSUM"))
        ps_nm = ctx.enter_context(tc.tile_pool(name="ps_nm", bufs=1, space="PSUM"))
        ps_i = ctx.enter_context(tc.tile_pool(name="ps_i", bufs=1, space="PSUM"))

        # ---- constants ----
        tri = const.tile([P, P], f32r)
        nc.sync.dma_start(out=tri, in_=tri_d[:, :])
        oz2 = const.tile([P, 2], bf16)
        nc.sync.dma_start(out=oz2, in_=oz2_d[:, :])
        zo2 = const.tile([P, 2], bf16)
        nc.sync.dma_start(out=zo2, in_=zo2_d[:, :])
        onesb = const.tile([1, P], f32r)
        nc.sync.dma_start(out=onesb, in_=onesb_d[:, :])
        invcnt = const.tile([P, ROWS], f32)
        nc.sync.dma_start(out=invcnt, in_=invcnt_d[:, :])
        w_sb = const.tile([P, NH], f32)
        for h in range(NH):
            nc.sync.dma_start(out=w_sb[:, h : h + 1], in_=wvec[h * P : (h + 1) * P, 0:1])
        if with_bias:
            b_sb = const.tile([P, NH], f32)
            for h in range(NH):
                nc.sync.dma_start(
                    out=b_sb[:, h : h + 1], in_=bvec[h * P : (h + 1) * P, 0:1]
                )
        eps_sb = const.tile([P, 1], f32)
        nc.vector.memset(eps_sb, EPS)

        # ---- persistent stat surfaces ----
        scombo = persist.tile([P, 2, ROWS], bf16)   # [:, 0, :]=s  [:, 1, :]=sq
        s_cs = persist.tile([P, ROWS], f32)         # prefix sums (within-row)
        sq_cs = persist.tile([P, ROWS], f32)
        mean_t = persist.tile([P, ROWS], f32)
        ex2_t = persist.tile([P, ROWS], f32)        # E[x^2] -> var
        msq_t = persist.tile([P, ROWS], f32)        # mean^2 -> sqrt(var+eps)
        istd_t = persist.tile([P, ROWS], f32)
        nm_t = persist.tile([P, ROWS], f32)         # -mean
        st_sb = persist.tile([P, 2], f32r)          # per-row totals (s, sq)

        xtiles = {}

        def phase_a(tix):
            t0 = tix * CHUNK
            xb = xpool.tile([P, NH, CHUNK], bf16, tag="x", name="xb")
            nc.gpsimd.dma_start(out=xb, in_=x_v[:, :, t0 : t0 + CHUNK])
            xtiles[tix] = xb
            xx = sqpool.tile([P, NH, CHUNK], bf16, tag="xx", name="xx")
            nc.vector.tensor_tensor(xx, xb, xb, ALU.mult)
            for a2 in range(2):  # 1000-col groups
                sp = ps_stat.tile([2, 2, 512], f32, tag="stat", name="sp")
                for j in range(2):
                    cs = slice((2 * a2 + j) * PB, (2 * a2 + j + 1) * PB)
                    # row 0 accumulates s (oz2 = [1,0] columns),
                    # row 1 accumulates sq (zo2 = [0,1]).
                    nc.tensor.matmul(sp[:, j, 0:PB], oz2, xb[:, 0, cs],
                                     start=True, stop=False)
                    nc.tensor.matmul(sp[:, j, 0:PB], oz2, xb[:, 1, cs],
                                     start=False, stop=False)
                    nc.tensor.matmul(sp[:, j, 0:PB], zo2, xx[:, 0, cs],
                                     start=False, stop=False)
                    nc.tensor.matmul(sp[:, j, 0:PB], zo2, xx[:, 1, cs],
                                     start=False, stop=True)
                rowt = rowpool.tile([2, 2, PB], bf16, tag="rowt", name="rowt")
                nc.scalar.copy(rowt, sp[:, :, 0:PB])
                # rows 16*tix+8*a2 .. +8 of the stat layout, s then sq.
                rp = 16 * tix + 8 * a2
                nc.sync.dma_start(
                    out=scombo[rp : rp + 8, 0, :], in_=rowt[0:1, :, :]
                )
                nc.sync.dma_start(
                    out=scombo[rp : rp + 8, 1, :], in_=rowt[1:2, :, :]
                )

        def stats():
            sv = scombo[:, 0, :]
            qv = scombo[:, 1, :]
            nc.vector.tensor_tensor_scan(
                out=s_cs, data0=sv, data1=sv, initial=0.0,
                op0=ALU.add, op1=ALU.bypass,
            )
            nc.vector.tensor_tensor_scan(
                out=sq_cs, data0=qv, data1=qv, initial=0.0,
                op0=ALU.add, op1=ALU.bypass,
            )
            nc.vector.tensor_copy(st_sb[:, 0:1], s_cs[:, ROWS - 1 : ROWS])
            nc.vector.tensor_copy(st_sb[:, 1:2], sq_cs[:, ROWS - 1 : ROWS])
            offps = ps_stat.tile([P, 2], f32, tag="stat", name="offps")
            nc.tensor.matmul(offps, tri, st_sb, start=True, stop=True)
            nc.vector.scalar_tensor_tensor(
                out=mean_t, in0=s_cs, scalar=offps[:, 0:1],
                in1=invcnt, op0=ALU.add, op1=ALU.mult,
            )
            nc.vector.scalar_tensor_tensor(
                out=ex2_t, in0=sq_cs, scalar=offps[:, 1:2],
                in1=invcnt, op0=ALU.add, op1=ALU.mult,
            )
            nc.vector.tensor_scalar_mul(nm_t, mean_t, -1.0)
            nc.vector.tensor_tensor(msq_t, mean_t, mean_t, ALU.mult)
            nc.vector.tensor_tensor(ex2_t, ex2_t, msq_t, ALU.subtract)
            nc.scalar.activation(msq_t, ex2_t, ACTF.Sqrt, bias=eps_sb, scale=1.0)
            nc.vector.reciprocal(out=istd_t, in_=msq_t)

        def phase_c(tix):
            t0 = tix * CHUNK
            xb = xtiles.pop(tix)
            rsl = slice(16 * tix, 16 * tix + 16)
            nmrow = browp.tile([1, CHUNK], f32, tag="brow", name="nmrow")
            nc.sync.dma_start(out=nmrow, in_=nm_t[rsl, :])
            isrow = browp.tile([1, CHUNK], f32, tag="brow", name="isrow")
            nc.sync.dma_start(out=isrow, in_=istd_t[rsl, :])
            y_t = ypool.tile([P, NH, CHUNK], bf16, tag="y", name="y_t")
            for half in range(2):  # 1000-col halves
                nmps = ps_nm.tile([P, 2, 512], f32, tag="nm", name="nmps")
                isps = ps_i.tile([P, 2, 512], f32, tag="ibc", name="isps")
                for j in range(2):
                    cs = slice((2 * half + j) * PB, (2 * half + j + 1) * PB)
                    nc.tensor.matmul(nmps[:, j, 0:PB], onesb,
                                     nmrow[0:1, cs].bitcast(f32r),
                                     start=True, stop=True)
                    nc.tensor.matmul(isps[:, j, 0:PB], onesb,
                                     isrow[0:1, cs].bitcast(f32r),
                                     start=True, stop=True)
                nm_sb = bcpool.tile([P, 2, PB], bf16, tag="bc", name="nm_sb")
                nc.scalar.copy(nm_sb, nmps[:, :, 0:PB])
                is_sb = bcpool.tile([P, 2, PB], bf16, tag="bc", name="is_sb")
                nc.scalar.copy(is_sb, isps[:, :, 0:PB])
                ccol = slice(half * 1000, (half + 1) * 1000)
                for h in range(NH):
                    x_ap = xb[:, h, ccol].rearrange("p (j n) -> p j n", j=2)
                    z = zpool.tile([P, 2, PB], bf16, tag="z", name="z")
                    nc.gpsimd.tensor_tensor(z, x_ap, nm_sb, ALU.add)
                    # y = (z * w[p]) * istd
                    nc.vector.scalar_tensor_tensor(
                        out=y_t[:, h, ccol].rearrange("p (j n) -> p j n", j=2),
                        in0=z, scalar=w_sb[:, h : h + 1], in1=is_sb,
                        op0=ALU.mult, op1=ALU.mult,
                    )
                    if with_bias:
                        ys = y_t[:, h, ccol].rearrange("p (j n) -> p j n", j=2)
                        nc.vector.tensor_scalar_add(
                            out=ys, in0=ys, scalar1=b_sb[:, h : h + 1]
                        )
            nc.gpsimd.dma_start(out=y_v[:, :, t0 : t0 + CHUNK], in_=y_t)

        for tix in range(NCHUNK):
            phase_a(tix)
        stats()
        for tix in range(NCHUNK):
            phase_c(tix)
    nc.compile()
    return nc


def _consts():
    tri = np.triu(np.ones((P, P), dtype=np.float32), k=1)  # tri[k,m]=1 iff k<m
    oz2 = np.zeros((P, 2), dtype=np.float32)
    oz2[:, 0] = 1.0
    zo2 = np.zeros((P, 2), dtype=np.float32)
    zo2[:, 1] = 1.0
    onesb = np.ones((1, P), dtype=np.float32)
    t_idx = (ROWS * np.arange(P, dtype=np.float64)[:, None]
             + np.arange(ROWS, dtype=np.float64)[None, :])
    invcnt = (1.0 / (C * (t_idx + 1.0))).astype(np.float32)
    return {"tri": tri, "oz2": oz2.astype(ml_dtypes.bfloat16),
            "zo2": zo2.astype(ml_dtypes.bfloat16), "onesb": onesb,
            "invcnt": invcnt}


def _get_nc(with_bias: bool):
    key = ("nc", with_bias)
    if key not in _cached:
        _cached[key] = _build_nc(with_bias)
    return _cached[key]


def _run(x, weight, bias, trace=False):
    from concourse.bass_utils import run_bass_kernel_spmd

    x = np.ascontiguousarray(np.asarray(x, dtype=np.float32))
    weight = np.asarray(weight, dtype=np.float32).reshape(C, 1)
    bias = np.asarray(bias, dtype=np.float32).reshape(C, 1)
    with_bias = bool(np.any(bias))
    nc = _get_nc(with_bias)

    consts = _consts()
    in_maps = []
    for b in range(B):
        m = {"x": np.ascontiguousarray(x[b]), "wvec": weight}
        if with_bias:
            m["bvec"] = bias
        m.update(consts)
        in_maps.append(m)

    res = run_bass_kernel_spmd(nc, in_maps, core_ids=list(range(B)), trace=trace)
    y = np.stack([r["y"] for r in res.results], axis=0)
    return y, res


def kernel(x, weight, bias):
    y, _ = _run(x, weight, bias, trace=False)
    return y


# revision 11
# speedup vs baseline: 1.5198x; 1.4082x over previous
"""Cumulative LayerNorm Trainium2 Bass kernel (v2: single-stats, bf16 path).

x: [B=8, C=256, T=16000] f32.  Per timestep t: normalize x[:, :, t] by the
mean/std of all elements x[:, :, t'<=t] (cumulative over channels+time), then
scale by weight[c] and add bias[c].

Sharding: pure data parallel over B across 8 NeuronCores (1 sample/core).

Per-core plan (C=256 = 2 halves of 128 partitions, T on the free dim).
All 8 io-tiles of x stay resident in SBUF as bf16 (64 KB/partition), so the
kernel runs as two DMA-bound epochs with one tiny serial stats step between:

  Phase A (x8 io-tiles of 2000 cols):
    - gpsimd cast-DMA loads x f32 HBM -> bf16 SBUF (one DMA per tile).
    - xx = x^2 (DVE, bf16 2x mode, one FD-4000 op per tile).
    - PE: per 500-col block one accumulation group of 4 bf16 matmuls with
      half-zero weights ([1100] / [0011] columns) sums s over both channel
      halves into PSUM rows 0-1 and sq into rows 2-3.
    - ACT evacuates [4, 1000] PSUM -> bf16, one DMA scatters it into the
      [128, 250] stat surface (t = 125*p + i; s in cols 0:125, sq 125:250).
  Stats (once):
    - DVE tensor_tensor_scan over the full [128, 125] stat rows (fp32
      accumulator over bf16 terms), strict-upper-triangular f32r matmul for
      exclusive cross-partition offsets, then mean / var / istd / -mean.
  Phase C (x8 io-tiles, per 1000-col half):
    - one [1, 2000] gather DMA per field per tile back into row layout,
      PE rank-1 broadcasts (ones x row) into PSUM, ACT evacuates to bf16.
    - z = x + (-mean) on GPSIMD; y = (z * w[p]) * istd on DVE (bf16 2x).
    - gpsimd cast-DMA store bf16 SBUF -> f32 HBM (one DMA per tile).
"""
import ml_dtypes
import numpy as np

B, C, T = 8, 256, 16000
P = 128
NH = 2                     # channel halves
CHUNK = 2000               # t per io-tile
NCHUNK = T // CHUNK        # 8
ROWS = T // P              # 125  (stat layout free dim; t = 125*p + i)
PB = 500                   # psum block columns
EPS = 1e-06

_cached = {}


def _build_nc(with_bias: bool):
    from contextlib import ExitStack

    import concourse.tile as tile
    from concourse import bacc, mybir

    f32 = mybir.dt.float32
    f32r = mybir.dt.float32r
    bf16 = mybir.dt.bfloat16
    ALU = mybir.AluOpType
    ACTF = mybir.ActivationFunctionType

    nc = bacc.Bacc()

    x = nc.dram_tensor("x", [C, T], f32, kind="ExternalInput")
    wvec = nc.dram_tensor("wvec", [C, 1], f32, kind="ExternalInput")
    tri_d = nc.dram_tensor("tri", [P, P], f32r, kind="ExternalInput")
    oz2_d = nc.dram_tensor("oz2", [P, 2], bf16, kind="ExternalInput")
    zo2_d = nc.dram_tensor("zo2", [P, 2], bf16, kind="ExternalInput")
    onesb_d = nc.dram_tensor("onesb", [1, P], f32r, kind="ExternalInput")
    invcnt_d = nc.dram_tensor("invcnt", [P, ROWS], f32, kind="ExternalInput")
    if with_bias:
        bvec = nc.dram_tensor("bvec", [C, 1], f32, kind="ExternalInput")
    y = nc.dram_tensor("y", [C, T], f32, kind="ExternalOutput")

    # DRAM views with channel halves split onto the partition dim.
    x_v = x.rearrange("(h p) t -> p h t", h=NH)
    y_v = y.rearrange("(h p) t -> p h t", h=NH)

    with tile.TileContext(nc) as tc, ExitStack() as ctx:
        const = ctx.enter_context(tc.tile_pool(name="const", bufs=1))
        persist = ctx.enter_context(tc.tile_pool(name="persist", bufs=1))
        xpool = ctx.enter_context(tc.tile_pool(name="xpool", bufs=NCHUNK))
        sqpool = ctx.enter_context(tc.tile_pool(name="sqpool", bufs=2))
        rowpool = ctx.enter_context(tc.tile_pool(name="rowpool", bufs=3))
        browp = ctx.enter_context(tc.tile_pool(name="browp", bufs=4))
        bcpool = ctx.enter_context(tc.tile_pool(name="bcpool", bufs=4))
        zpool = ctx.enter_context(tc.tile_pool(name="zpool", bufs=4))
        ypool = ctx.enter_context(tc.tile_pool(name="ypool", bufs=2))
        ps_stat = ctx.enter_context(tc.tile_pool(name="ps_stat", bufs=2, space="PSUM"))
        ps_nm = ctx.enter_context(tc.tile_pool(name="ps_nm", bufs=1, space="PSUM"))
        ps_i = ctx.enter_context(tc.tile_pool(name="ps_i", bufs=1, space="PSUM"))

        # ---- constants ----
        tri = const.tile([P, P], f32r)
        nc.sync.dma_start(out=tri, in_=tri_d[:, :])
        oz2 = const.tile([P, 2], bf16)
        nc.sync.dma_start(out=oz2, in_=oz2_d[:, :])
        zo2 = const.tile([P, 2], bf16)
        nc.sync.dma_start(out=zo2, in_=zo2_d[:, :])
        onesb = const.tile([1, P], f32r)
        nc.sync.dma_start(out=onesb, in_=onesb_d[:, :])
        invcnt = const.tile([P, ROWS], f32)
        nc.sync.dma_start(out=invcnt, in_=invcnt_d[:, :])
        w_sb = const.tile([P, NH], f32)
        for h in range(NH):
            nc.sync.dma_start(out=w_sb[:, h : h + 1], in_=wvec[h * P : (h + 1) * P, 0:1])
        if with_bias:
            b_sb = const.tile([P, NH], f32)
            for h in range(NH):
                nc.sync.dma_start(
                    out=b_sb[:, h : h + 1], in_=bvec[h * P : (h + 1) * P, 0:1]
                )
        eps_sb = const.tile([P, 1], f32)
        nc.vector.memset(eps_sb, EPS)

        # ---- persistent stat surfaces ----
        scombo = persist.tile([P, 2, ROWS], bf16)   # [:, 0, :]=s  [:, 1, :]=sq
        s_cs = persist.tile([P, ROWS], f32)         # prefix sums (within-row)
        sq_cs = persist.tile([P, ROWS], f32)
        mean_t = persist.tile([P, ROWS], f32)
        ex2_t = persist.tile([P, ROWS], f32)        # E[x^2] -> var
        msq_t = persist.tile([P, ROWS], f32)        # mean^2 -> sqrt(var+eps)
        istd_t = persist.tile([P, ROWS], f32)
        nm_t = persist.tile([P, ROWS], f32)         # -mean
        st_sb = persist.tile([P, 2], f32r)          # per-row totals (s, sq)

        xtiles = {}

        def phase_a(tix):
            t0 = tix * CHUNK
            xb = xpool.tile([P, NH, CHUNK], bf16, tag="x", name="xb")
            nc.gpsimd.dma_start(out=xb, in_=x_v[:, :, t0 : t0 + CHUNK])
            xtiles[tix] = xb
            xx = sqpool.tile([P, NH, CHUNK], bf16, tag="xx", name="xx")
            nc.vector.tensor_tensor(xx, xb, xb, ALU.mult)
            for a2 in range(2):  # 1000-col groups
                sp = ps_stat.tile([2, 2, 512], f32, tag="stat", name="sp")
                for j in range(2):
                    cs = slice((2 * a2 + j) * PB, (2 * a2 + j + 1) * PB)
                    # row 0 accumulates s (oz2 = [1,0] columns),
                    # row 1 accumulates sq (zo2 = [0,1]).
                    nc.tensor.matmul(sp[:, j, 0:PB], oz2, xb[:, 0, cs],
                                     start=True, stop=False)
                    nc.tensor.matmul(sp[:, j, 0:PB], oz2, xb[:, 1, cs],
                                     start=False, stop=False)
                    nc.tensor.matmul(sp[:, j, 0:PB], zo2, xx[:, 0, cs],
                                     start=False, stop=False)
                    nc.tensor.matmul(sp[:, j, 0:PB], zo2, xx[:, 1, cs],
                                     start=False, stop=True)
                rowt = rowpool.tile([2, 2, PB], bf16, tag="rowt", name="rowt")
                nc.scalar.copy(rowt, sp[:, :, 0:PB])
                # rows 16*tix+8*a2 .. +8 of the stat layout, s then sq.
                rp = 16 * tix + 8 * a2
                nc.sync.dma_start(
                    out=scombo[rp : rp + 8, 0, :], in_=rowt[0:1, :, :]
                )
                nc.sync.dma_start(
                    out=scombo[rp : rp + 8, 1, :], in_=rowt[1:2, :, :]
                )

        def stats():
            sv = scombo[:, 0, :]
            qv = scombo[:, 1, :]
            nc.vector.tensor_tensor_scan(
                out=s_cs, data0=sv, data1=sv, initial=0.0,
                op0=ALU.add, op1=ALU.bypass,
            )
            nc.vector.tensor_tensor_scan(
                out=sq_cs, data0=qv, data1=qv, initial=0.0,
                op0=ALU.add, op1=ALU.bypass,
            )
            nc.vector.tensor_copy(st_sb[:, 0:1], s_cs[:, ROWS - 1 : ROWS])
            nc.vector.tensor_copy(st_sb[:, 1:2], sq_cs[:, ROWS - 1 : ROWS])
            offps = ps_stat.tile([P, 2], f32, tag="stat", name="offps")
            nc.tensor.matmul(offps, tri, st_sb, start=True, stop=True)
            nc.vector.scalar_tensor_tensor(
                out=mean_t, in0=s_cs, scalar=offps[:, 0:1],
                in1=invcnt, op0=ALU.add, op1=ALU.mult,
            )
            nc.vector.scalar_tensor_tensor(
                out=ex2_t, in0=sq_cs, scalar=offps[:, 1:2],
                in1=invcnt, op0=ALU.add, op1=ALU.mult,
            )
            nc.vector.tensor_scalar_mul(nm_t, mean_t, -1.0)
            nc.vector.tensor_tensor(msq_t, mean_t, mean_t, ALU.mult)
            nc.vector.tensor_tensor(ex2_t, ex2_t, msq_t, ALU.subtract)
            nc.scalar.activation(msq_t, ex2_t, ACTF.Sqrt, bias=eps_sb, scale=1.0)
            nc.vector.reciprocal(out=istd_t, in_=msq_t)

        def phase_c(tix):
            t0 = tix * CHUNK
            xb = xtiles.pop(tix)
            rsl = slice(16 * tix, 16 * tix + 16)
            nmrow = browp.tile([1, CHUNK], f32, tag="brow", name="nmrow")
            nc.sync.dma_start(out=nmrow, in_=nm_t[rsl, :])
            isrow = browp.tile([1, CHUNK], f32, tag="brow", name="isrow")
            nc.sync.dma_start(out=isrow, in_=istd_t[rsl, :])
            y_t = ypool.tile([P, NH, CHUNK], bf16, tag="y", name="y_t")
            for half in range(2):  # 1000-col halves
                nmps = ps_nm.tile([P, 2, 512], f32, tag="nm", name="nmps")
                isps = ps_i.tile([P, 2, 512], f32, tag="ibc", name="isps")
                for j in range(2):
                    cs = slice((2 * half + j) * PB, (2 * half + j + 1) * PB)
                    nc.tensor.matmul(nmps[:, j, 0:PB], onesb,
                                     nmrow[0:1, cs].bitcast(f32r),
                                     start=True, stop=True)
                    nc.tensor.matmul(isps[:, j, 0:PB], onesb,
                                     isrow[0:1, cs].bitcast(f32r),
                                     start=True, stop=True)
                nm_sb = bcpool.tile([P, 2, PB], bf16, tag="bc", name="nm_sb")
                nc.scalar.copy(nm_sb, nmps[:, :, 0:PB])
                is_sb = bcpool.tile([P, 2, PB], bf16, tag="bc", name="is_sb")
                nc.scalar.copy(is_sb, isps[:, :, 0:PB])
                ccol = slice(half * 1000, (half + 1) * 1000)
                for h in range(NH):
                    x_ap = xb[:, h, ccol].rearrange("p (j n) -> p j n", j=2)
                    ys = y_t[:, h, ccol].rearrange("p (j n) -> p j n", j=2)
                    z = zpool.tile([P, 2, PB], bf16, tag="z", name="z")
                    nc.vector.tensor_tensor(z, x_ap, nm_sb, ALU.add)
                    u = zpool.tile([P, 2, PB], bf16, tag="z", name="u")
                    nc.vector.tensor_tensor(u, z, is_sb, ALU.mult)
                    # y = u * w[p]  (tensor_scalar runs 4x/cycle on bf16)
                    if with_bias:
                        nc.vector.tensor_scalar(
                            out=ys, in0=u, scalar1=w_sb[:, h : h + 1],
                            scalar2=b_sb[:, h : h + 1],
                            op0=ALU.mult, op1=ALU.add,
                        )
                    else:
                        nc.vector.tensor_scalar_mul(ys, u, w_sb[:, h : h + 1])
            nc.gpsimd.dma_start(out=y_v[:, :, t0 : t0 + CHUNK], in_=y_t)

        for tix in range(NCHUNK):
            phase_a(tix)
        stats()
        for tix in range(NCHUNK):
            phase_c(tix)
    nc.compile()
    return nc


def _consts():
    tri = np.triu(np.ones((P, P), dtype=np.float32), k=1)  # tri[k,m]=1 iff k<m
    oz2 = np.zeros((P, 2), dtype=np.float32)
    oz2[:, 0] = 1.0
    zo2 = np.zeros((P, 2), dtype=np.float32)
    zo2[:, 1] = 1.0
    onesb = np.ones((1, P), dtype=np.float32)
    t_idx = (ROWS * np.arange(P, dtype=np.float64)[:, None]
             + np.arange(ROWS, dtype=np.float64)[None, :])
    invcnt = (1.0 / (C * (t_idx + 1.0))).astype(np.float32)
    return {"tri": tri, "oz2": oz2.astype(ml_dtypes.bfloat16),
            "zo2": zo2.astype(ml_dtypes.bfloat16), "onesb": onesb,
            "invcnt": invcnt}


def _get_nc(with_bias: bool):
    key = ("nc", with_bias)
    if key not in _cached:
        _cached[key] = _build_nc(with_bias)
    return _cached[key]


def _run(x, weight, bias, trace=False):
    from concourse.bass_utils import run_bass_kernel_spmd

    x = np.ascontiguousarray(np.asarray(x, dtype=np.float32))
    weight = np.asarray(weight, dtype=np.float32).reshape(C, 1)
    bias = np.asarray(bias, dtype=np.float32).reshape(C, 1)
    with_bias = bool(np.any(bias))
    nc = _get_nc(with_bias)

    consts = _consts()
    in_maps = []
    for b in range(B):
        m = {"x": np.ascontiguousarray(x[b]), "wvec": weight}
        if with_bias:
            m["bvec"] = bias
        m.update(consts)
        in_maps.append(m)

    res = run_bass_kernel_spmd(nc, in_maps, core_ids=list(range(B)), trace=trace)
    y = np.stack([r["y"] for r in res.results], axis=0)
    return y, res


def kernel(x, weight, bias):
    y, _ = _run(x, weight, bias, trace=False)
    return y


# revision 17
# speedup vs baseline: 1.6066x; 1.0571x over previous
"""Cumulative LayerNorm Trainium2 Bass kernel (v2: single-stats, bf16 path).

x: [B=8, C=256, T=16000] f32.  Per timestep t: normalize x[:, :, t] by the
mean/std of all elements x[:, :, t'<=t] (cumulative over channels+time), then
scale by weight[c] and add bias[c].

Sharding: pure data parallel over B across 8 NeuronCores (1 sample/core).

Per-core plan (C=256 = 2 halves of 128 partitions, T on the free dim).
All 8 io-tiles of x stay resident in SBUF as bf16 (64 KB/partition), so the
kernel runs as two DMA-bound epochs with one tiny serial stats step between:

  Phase A (x8 io-tiles of 2000 cols):
    - gpsimd cast-DMA loads x f32 HBM -> bf16 SBUF (one DMA per tile).
    - xx = x^2 (DVE, bf16 2x mode, one FD-4000 op per tile).
    - PE: per 500-col block one accumulation group of 4 bf16 matmuls with
      half-zero weights ([1100] / [0011] columns) sums s over both channel
      halves into PSUM rows 0-1 and sq into rows 2-3.
    - ACT evacuates [4, 1000] PSUM -> bf16, one DMA scatters it into the
      [128, 250] stat surface (t = 125*p + i; s in cols 0:125, sq 125:250).
  Stats (once):
    - DVE tensor_tensor_scan over the full [128, 125] stat rows (fp32
      accumulator over bf16 terms), strict-upper-triangular f32r matmul for
      exclusive cross-partition offsets, then mean / var / istd / -mean.
  Phase C (x8 io-tiles, per 1000-col half):
    - one [1, 2000] gather DMA per field per tile back into row layout,
      PE rank-1 broadcasts (ones x row) into PSUM, ACT evacuates to bf16.
    - z = x + (-mean) on GPSIMD; y = (z * w[p]) * istd on DVE (bf16 2x).
    - gpsimd cast-DMA store bf16 SBUF -> f32 HBM (one DMA per tile).
"""
import ml_dtypes
import numpy as np

B, C, T = 8, 256, 16000
P = 128
NH = 2                     # channel halves
CHUNK = 2000               # t per io-tile
NCHUNK = T // CHUNK        # 8
ROWS = T // P              # 125  (stat layout free dim; t = 125*p + i)
PB = 500                   # psum block columns
EPS = 1e-06

_cached = {}


def _build_nc(with_bias: bool):
    from contextlib import ExitStack

    import concourse.tile as tile
    from concourse import bacc, mybir

    f32 = mybir.dt.float32
    f32r = mybir.dt.float32r
    bf16 = mybir.dt.bfloat16
    ALU = mybir.AluOpType
    ACTF = mybir.ActivationFunctionType

    nc = bacc.Bacc()

    x = nc.dram_tensor("x", [C, T], f32, kind="ExternalInput")
    wvec = nc.dram_tensor("wvec", [C, 1], f32, kind="ExternalInput")
    tri_d = nc.dram_tensor("tri", [P, P], f32r, kind="ExternalInput")
    oz2_d = nc.dram_tensor("oz2", [P, 2], bf16, kind="ExternalInput")
    zo2_d = nc.dram_tensor("zo2", [P, 2], bf16, kind="ExternalInput")
    onesb_d = nc.dram_tensor("onesb", [1, P], f32r, kind="ExternalInput")
    invcnt_d = nc.dram_tensor("invcnt", [P, ROWS], f32, kind="ExternalInput")
    if with_bias:
        bvec = nc.dram_tensor("bvec", [C, 1], f32, kind="ExternalInput")
    y = nc.dram_tensor("y", [C, T], f32, kind="ExternalOutput")

    # DRAM views with channel halves split onto the partition dim.
    x_v = x.rearrange("(h p) t -> p h t", h=NH)
    y_v = y.rearrange("(h p) t -> p h t", h=NH)

    with tile.TileContext(nc) as tc, ExitStack() as ctx:
        const = ctx.enter_context(tc.tile_pool(name="const", bufs=1))
        persist = ctx.enter_context(tc.tile_pool(name="persist", bufs=1))
        xpool = ctx.enter_context(tc.tile_pool(name="xpool", bufs=NCHUNK))
        sqpool = ctx.enter_context(tc.tile_pool(name="sqpool", bufs=2))
        rowpool = ctx.enter_context(tc.tile_pool(name="rowpool", bufs=3))
        browp = ctx.enter_context(tc.tile_pool(name="browp", bufs=4))
        bcpool = ctx.enter_context(tc.tile_pool(name="bcpool", bufs=4))
        zpool = ctx.enter_context(tc.tile_pool(name="zpool", bufs=4))
        ypool = ctx.enter_context(tc.tile_pool(name="ypool", bufs=2))
        ps_stat = ctx.enter_context(tc.tile_pool(name="ps_stat", bufs=2, space="PSUM"))
        ps_nm = ctx.enter_context(tc.tile_pool(name="ps_nm", bufs=1, space="PSUM"))
        ps_i = ctx.enter_context(tc.tile_pool(name="ps_i", bufs=1, space="PSUM"))

        # ---- constants ----
        tri = const.tile([P, P], f32r)
        nc.sync.dma_start(out=tri, in_=tri_d[:, :])
        oz2 = const.tile([P, 2], bf16)
        nc.sync.dma_start(out=oz2, in_=oz2_d[:, :])
        zo2 = const.tile([P, 2], bf16)
        nc.sync.dma_start(out=zo2, in_=zo2_d[:, :])
        onesb = const.tile([1, P], f32r)
        nc.sync.dma_start(out=onesb, in_=onesb_d[:, :])
        invcnt = const.tile([P, ROWS], f32)
        nc.sync.dma_start(out=invcnt, in_=invcnt_d[:, :])
        w_sb = const.tile([P, NH], f32)
        for h in range(NH):
            nc.sync.dma_start(out=w_sb[:, h : h + 1], in_=wvec[h * P : (h + 1) * P, 0:1])
        if with_bias:
            b_sb = const.tile([P, NH], f32)
            for h in range(NH):
                nc.sync.dma_start(
                    out=b_sb[:, h : h + 1], in_=bvec[h * P : (h + 1) * P, 0:1]
                )
        eps_sb = const.tile([P, 1], f32)
        nc.vector.memset(eps_sb, EPS)

        # ---- persistent stat surfaces ----
        scombo = persist.tile([P, 2, ROWS], bf16)   # [:, 0, :]=s  [:, 1, :]=sq
        s_cs = persist.tile([P, ROWS], f32)         # prefix sums (within-row)
        sq_cs = persist.tile([P, ROWS], f32)
        mean_t = persist.tile([P, ROWS], f32)
        ex2_t = persist.tile([P, ROWS], f32)        # E[x^2] -> var
        msq_t = persist.tile([P, ROWS], f32)        # mean^2 -> sqrt(var+eps)
        istd_t = persist.tile([P, ROWS], f32)
        nm_t = persist.tile([P, ROWS], f32)         # -mean
        st_sb = persist.tile([P, 2], f32r)          # per-row totals (s, sq)
        # Zeroed so the half-0 triangular matmul reads 0 (not garbage) in the
        # not-yet-written rows 64..127 (tri zeros them, but 0*NaN = NaN).
        nc.vector.memset(st_sb.bitcast(f32), 0.0)

        xtiles = {}

        def phase_a(tix):
            t0 = tix * CHUNK
            xb = xpool.tile([P, NH, CHUNK], bf16, tag="x", name="xb")
            nc.gpsimd.dma_start(out=xb, in_=x_v[:, :, t0 : t0 + CHUNK])
            xtiles[tix] = xb
            xx = sqpool.tile([P, NH, CHUNK], bf16, tag="xx", name="xx")
            nc.vector.tensor_tensor(xx, xb, xb, ALU.mult)
            for a2 in range(2):  # 1000-col groups
                sp = ps_stat.tile([2, 2, 512], f32, tag="stat", name="sp")
                for j in range(2):
                    cs = slice((2 * a2 + j) * PB, (2 * a2 + j + 1) * PB)
                    # row 0 accumulates s (oz2 = [1,0] columns),
                    # row 1 accumulates sq (zo2 = [0,1]).
                    nc.tensor.matmul(sp[:, j, 0:PB], oz2, xb[:, 0, cs],
                                     start=True, stop=False)
                    nc.tensor.matmul(sp[:, j, 0:PB], oz2, xb[:, 1, cs],
                                     start=False, stop=False)
                    nc.tensor.matmul(sp[:, j, 0:PB], zo2, xx[:, 0, cs],
                                     start=False, stop=False)
                    nc.tensor.matmul(sp[:, j, 0:PB], zo2, xx[:, 1, cs],
                                     start=False, stop=True)
                rowt = rowpool.tile([2, 2, PB], bf16, tag="rowt", name="rowt")
                nc.scalar.copy(rowt, sp[:, :, 0:PB])
                # rows 16*tix+8*a2 .. +8 of the stat layout, s then sq.
                rp = 16 * tix + 8 * a2
                nc.sync.dma_start(
                    out=scombo[rp : rp + 8, 0, :], in_=rowt[0:1, :, :]
                )
                nc.sync.dma_start(
                    out=scombo[rp : rp + 8, 1, :], in_=rowt[1:2, :, :]
                )

        def stats(hf):
            """Prefix stats for stat-layout rows 64*hf .. 64*hf+64 (io-tiles
            4*hf .. 4*hf+4).  The tri matmul contracts all 128 st_sb rows;
            future rows are zero so the exclusive offsets stay exact."""
            sl = slice(64 * hf, 64 * hf + 64)
            sv = scombo[sl, 0, :]
            qv = scombo[sl, 1, :]
            nc.vector.tensor_tensor_scan(
                out=s_cs[sl, :], data0=sv, data1=sv, initial=0.0,
                op0=ALU.add, op1=ALU.bypass,
            )
            nc.vector.tensor_tensor_scan(
                out=sq_cs[sl, :], data0=qv, data1=qv, initial=0.0,
                op0=ALU.add, op1=ALU.bypass,
            )
            nc.vector.tensor_copy(st_sb[sl, 0:1], s_cs[sl, ROWS - 1 : ROWS])
            nc.vector.tensor_copy(st_sb[sl, 1:2], sq_cs[sl, ROWS - 1 : ROWS])
            offps = ps_stat.tile([P, 2], f32, tag="stat", name="offps")
            nc.tensor.matmul(offps, tri, st_sb, start=True, stop=True)
            nc.vector.scalar_tensor_tensor(
                out=mean_t[sl, :], in0=s_cs[sl, :], scalar=offps[sl, 0:1],
                in1=invcnt[sl, :], op0=ALU.add, op1=ALU.mult,
            )
            nc.vector.scalar_tensor_tensor(
                out=ex2_t[sl, :], in0=sq_cs[sl, :], scalar=offps[sl, 1:2],
                in1=invcnt[sl, :], op0=ALU.add, op1=ALU.mult,
            )
            nc.vector.tensor_scalar_mul(nm_t[sl, :], mean_t[sl, :], -1.0)
            nc.vector.tensor_tensor(msq_t[sl, :], mean_t[sl, :], mean_t[sl, :],
                                    ALU.mult)
            nc.vector.tensor_tensor(ex2_t[sl, :], ex2_t[sl, :], msq_t[sl, :],
                                    ALU.subtract)
            nc.scalar.activation(msq_t[sl, :], ex2_t[sl, :], ACTF.Sqrt,
                                 bias=eps_sb[sl, :], scale=1.0)
            nc.vector.reciprocal(out=istd_t[sl, :], in_=msq_t[sl, :])

        def phase_c(tix):
            t0 = tix * CHUNK
            xb = xtiles.pop(tix)
            rsl = slice(16 * tix, 16 * tix + 16)
            # gathers ride the scalar HWDGE queue so they can't head-of-line
            # block the stat scatters on sync while waiting for stats(hf)
            nmrow = browp.tile([1, CHUNK], f32, tag="brow", name="nmrow")
            nc.scalar.dma_start(out=nmrow, in_=nm_t[rsl, :])
            isrow = browp.tile([1, CHUNK], f32, tag="brow", name="isrow")
            nc.scalar.dma_start(out=isrow, in_=istd_t[rsl, :])
            y_t = ypool.tile([P, NH, CHUNK], bf16, tag="y", name="y_t")
            for half in range(2):  # 1000-col halves
                nmps = ps_nm.tile([P, 2, 512], f32, tag="nm", name="nmps")
                isps = ps_i.tile([P, 2, 512], f32, tag="ibc", name="isps")
                for j in range(2):
                    cs = slice((2 * half + j) * PB, (2 * half + j + 1) * PB)
                    nc.tensor.matmul(nmps[:, j, 0:PB], onesb,
                                     nmrow[0:1, cs].bitcast(f32r),
                                     start=True, stop=True)
                    nc.tensor.matmul(isps[:, j, 0:PB], onesb,
                                     isrow[0:1, cs].bitcast(f32r),
                                     start=True, stop=True)
                nm_sb = bcpool.tile([P, 2, PB], bf16, tag="bc", name="nm_sb")
                nc.scalar.copy(nm_sb, nmps[:, :, 0:PB])
                is_sb = bcpool.tile([P, 2, PB], bf16, tag="bc", name="is_sb")
                nc.scalar.copy(is_sb, isps[:, :, 0:PB])
                ccol = slice(half * 1000, (half + 1) * 1000)
                for h in range(NH):
                    x_ap = xb[:, h, ccol].rearrange("p (j n) -> p j n", j=2)
                    ys = y_t[:, h, ccol].rearrange("p (j n) -> p j n", j=2)
                    z = zpool.tile([P, 2, PB], bf16, tag="z", name="z")
                    nc.vector.tensor_tensor(z, x_ap, nm_sb, ALU.add)
                    u = zpool.tile([P, 2, PB], bf16, tag="z", name="u")
                    nc.vector.tensor_tensor(u, z, is_sb, ALU.mult)
                    # y = u * w[p]  (tensor_scalar runs 4x/cycle on bf16)
                    if with_bias:
                        nc.vector.tensor_scalar(
                            out=ys, in0=u, scalar1=w_sb[:, h : h + 1],
                            scalar2=b_sb[:, h : h + 1],
                            op0=ALU.mult, op1=ALU.add,
                        )
                    else:
                        nc.vector.tensor_scalar_mul(ys, u, w_sb[:, h : h + 1])
                nc.gpsimd.dma_start(
                    out=y_v[:, :, t0 + half * 1000 : t0 + (half + 1) * 1000],
                    in_=y_t[:, :, ccol],
                )

        # Emission: stats for rows 0-63 fire after io-tiles 0-3, so phase C
        # of tiles 0-3 (and their y stores) overlaps the tile 4-7 loads on
        # the HBM-saturated window; stats for rows 64-127 then cover the rest.
        for tix in range(5):
            phase_a(tix)
        stats(0)
        phase_c(0)
        phase_a(5)
        phase_c(1)
        phase_a(6)
        phase_c(2)
        phase_a(7)
        phase_c(3)
        stats(1)
        for tix in range(4, NCHUNK):
            phase_c(tix)
    nc.compile()
    return nc


def _consts():
    tri = np.triu(np.ones((P, P), dtype=np.float32), k=1)  # tri[k,m]=1 iff k<m
    oz2 = np.zeros((P, 2), dtype=np.float32)
    oz2[:, 0] = 1.0
    zo2 = np.zeros((P, 2), dtype=np.float32)
    zo2[:, 1] = 1.0
    onesb = np.ones((1, P), dtype=np.float32)
    t_idx = (ROWS * np.arange(P, dtype=np.float64)[:, None]
             + np.arange(ROWS, dtype=np.float64)[None, :])
    invcnt = (1.0 / (C * (t_idx + 1.0))).astype(np.float32)
    return {"tri": tri, "oz2": oz2.astype(ml_dtypes.bfloat16),
            "zo2": zo2.astype(ml_dtypes.bfloat16), "onesb": onesb,
            "invcnt": invcnt}


def _get_nc(with_bias: bool):
    key = ("nc", with_bias)
    if key not in _cached:
        _cached[key] = _build_nc(with_bias)
    return _cached[key]


def _run(x, weight, bias, trace=False):
    from concourse.bass_utils import run_bass_kernel_spmd

    x = np.ascontiguousarray(np.asarray(x, dtype=np.float32))
    weight = np.asarray(weight, dtype=np.float32).reshape(C, 1)
    bias = np.asarray(bias, dtype=np.float32).reshape(C, 1)
    with_bias = bool(np.any(bias))
    nc = _get_nc(with_bias)

    consts = _consts()
    in_maps = []
    for b in range(B):
        m = {"x": np.ascontiguousarray(x[b]), "wvec": weight}
        if with_bias:
            m["bvec"] = bias
        m.update(consts)
        in_maps.append(m)

    res = run_bass_kernel_spmd(nc, in_maps, core_ids=list(range(B)), trace=trace)
    y = np.stack([r["y"] for r in res.results], axis=0)
    return y, res


def kernel(x, weight, bias):
    y, _ = _run(x, weight, bias, trace=False)
    return y
